# revision 11
# baseline (speedup 1.0000x reference)
"""EMA dechunker kernel for Trainium2 (Bass/Tile), 8-core data-parallel.

Problem: for each batch row
  smoothed[j] = m[j] ? clip(p[j])*emb[j] + (1-clip(p[j]))*smoothed[j-1]
                     : smoothed[j-1]
  frames[l]   = smoothed[clip(cumsum(boundary)[l]-1, 0, J-1)]

Sharding: batch dim B=16 split across 8 cores (2 rows/core). Each core:
  1. coeffs: c = clip(conf)*mask, a = 1-c  (tiny row ops)
  2. EMA: PE-transpose emb blocks (scaled by diag(c)) into (D-part, J-free)
     layout, then one tensor_tensor_scan per (row, D-block) runs the whole
     first-order recurrence along the free dim. Transpose back, store
     smoothed to DRAM.
  3. idx: two-level cumsum of boundary mask (PE tri-matmul over 16
     partitions + free-dim scan of column sums), -1, clip, cast int16.
  4. gather: dma_gather pulls frames' source rows from DRAM smoothed,
     direct DMA writes the output.
"""

from contextlib import ExitStack

import numpy as np

import concourse.bass as bass
import concourse.tile as tile
from concourse import bacc, mybir
from concourse.bass_utils import run_bass_kernel_spmd
from concourse.masks import make_identity

F32 = mybir.dt.float32
I16 = mybir.dt.int16
U8 = mybir.dt.uint8
OP = mybir.AluOpType

B, J, L, D = 16, 1024, 4096, 512
N_CORES = 8
BL = B // N_CORES          # 2 batch rows per core
T = 128                    # j-chunk (partition) size
NCH = J // T               # 8 chunks per row
NDB = D // 128             # 4 D-blocks of 128 partitions
NSUB = 4                   # sub-gathers per row
SUBL = L // NSUB           # 1024 frames per sub-gather
EPS = 1e-4


def _body(tc, ctx):
    nc = tc.nc
    emb = nc.dram_tensor("unit_embeddings", [BL, J, D], F32, kind="ExternalInput").ap()
    conf = nc.dram_tensor("unit_confidence", [BL, J], F32, kind="ExternalInput").ap()
    mask = nc.dram_tensor("unit_mask", [BL, J], U8, kind="ExternalInput").ap()
    bdry = nc.dram_tensor("boundary_mask", [BL, L], U8, kind="ExternalInput").ap()
    out = nc.dram_tensor("frames", [BL, L, D], F32, kind="ExternalOutput").ap()

    const_p = ctx.enter_context(tc.tile_pool(name="const", bufs=1))
    coef_p = ctx.enter_context(tc.tile_pool(name="coef", bufs=1))
    et_p = ctx.enter_context(tc.tile_pool(name="et", bufs=3))
    etT_p = ctx.enter_context(tc.tile_pool(name="etT", bufs=BL))
    smT_p = ctx.enter_context(tc.tile_pool(name="smT", bufs=2 * NDB))
    smn_p = ctx.enter_context(tc.tile_pool(name="smn", bufs=3))
    diag_p = ctx.enter_context(tc.tile_pool(name="diag", bufs=4))
    idx_p = ctx.enter_context(tc.tile_pool(name="idx", bufs=1))
    gout_p = ctx.enter_context(tc.tile_pool(name="gout", bufs=2))
    dram_p = ctx.enter_context(tc.tile_pool(name="dram", bufs=1, space="DRAM"))
    psum_p = ctx.enter_context(tc.tile_pool(name="psum", bufs=7, space="PSUM"))

    ps_ctr = [0]

    def ps_tile(shape):
        ps_ctr[0] += 1
        return psum_p.tile(shape, F32, tag="ps", name=f"ps{ps_ctr[0]}")

    # --- constants ---
    ident = const_p.tile([128, 128], F32)
    make_identity(nc, ident[:])
    ones_row = const_p.tile([1, 128], F32)
    nc.gpsimd.memset(ones_row[:], 1.0)
    ones_col16 = const_p.tile([16, 1], F32)
    nc.gpsimd.memset(ones_col16[:], 1.0)
    zeros_row = const_p.tile([1, 256], F32)
    nc.gpsimd.memset(zeros_row[:], 0.0)
    # tri16[k, p] = 1 iff k <= p  (lhsT for partition-dim inclusive cumsum):
    # running-sum of the identity along the free dim.
    zeros16 = const_p.tile([16, 16], F32)
    nc.gpsimd.memset(zeros16[:], 0.0)
    tri16 = const_p.tile([16, 16], F32)
    nc.vector.tensor_tensor_scan(
        out=tri16[:], data0=zeros16[:], data1=ident[:16, :16],
        initial=0.0, op0=OP.add, op1=OP.add,
    )

    smoothed = [dram_p.tile([J, D], F32, name=f"smoothed{r}") for r in range(BL)]

    # --- phase 1: coefficients ---
    c_rows = []
    a_bc = []
    for r in range(BL):
        cf = coef_p.tile([1, J], F32, tag=f"cf{r}")
        nc.sync.dma_start(cf[:], conf[r : r + 1, :])
        mk = coef_p.tile([1, J], F32, tag=f"mk{r}")
        nc.gpsimd.dma_start(mk[:], mask[r : r + 1, :])  # u8 -> f32 cast in DMA
        c_r = coef_p.tile([1, J], F32, tag=f"c{r}")
        nc.vector.tensor_scalar(
            out=c_r[:], in0=cf[:], scalar1=EPS, scalar2=1.0 - EPS,
            op0=OP.max, op1=OP.min,
        )
        nc.vector.tensor_tensor(out=c_r[:], in0=c_r[:], in1=mk[:], op=OP.mult)
        a_r = coef_p.tile([1, J], F32, tag=f"a{r}")
        nc.vector.tensor_scalar(
            out=a_r[:], in0=c_r[:], scalar1=-1.0, scalar2=1.0,
            op0=OP.mult, op1=OP.add,
        )
        c_rows.append(c_r)
        # broadcast a to 128 partitions via K=1 matmul
        abc = coef_p.tile([128, J], F32, tag=f"abc{r}")
        for h in range(J // 512):
            pb = ps_tile([128, 512])
            nc.tensor.matmul(
                out=pb[:], lhsT=ones_row[:], rhs=a_r[:, h * 512 : (h + 1) * 512],
                start=True, stop=True,
            )
            nc.scalar.copy(abc[:, h * 512 : (h + 1) * 512], pb[:])
        a_bc.append(abc)

    # c columns: cstage[(r*8+g), :] = c_r[g*128:(g+1)*128] ; transpose -> (128, 16)
    cstage = coef_p.tile([2 * NCH, T], F32)
    for r in range(BL):
        nc.sync.dma_start(cstage[r * NCH : (r + 1) * NCH, :], c_rows[r][:])
    pc = ps_tile([128, 2 * NCH])
    nc.tensor.matmul(
        out=pc[:], lhsT=cstage[:], rhs=ident[: 2 * NCH, : 2 * NCH],
        start=True, stop=True,
    )
    c_cols = coef_p.tile([128, 2 * NCH], F32)
    nc.vector.tensor_copy(c_cols[:], pc[:])

    diag_c = {}
    for r in range(BL):
        for g in range(NCH):
            col = r * NCH + g
            dg = diag_p.tile([128, 128], F32, tag="diag", name=f"diag{r}_{g}")
            nc.vector.tensor_tensor(
                out=dg[:], in0=c_cols[:, col : col + 1].to_broadcast([128, 128]),
                in1=ident[:], op=OP.mult,
            )
            diag_c[(r, g)] = dg

    # --- phase 2: indices ---
    idx_rep = []
    for r in range(BL):
        bd_f = idx_p.tile([1, L], F32, tag=f"bdf{r}")
        nc.gpsimd.dma_start(bd_f[:], bdry[r : r + 1, :])  # u8 -> f32
        # W[p, q] = bd[q*16 + p] for p in [0,16), q in [0,256)
        w_sb = idx_p.tile([16, 256], F32, tag=f"w{r}")
        for h in range(2):
            vh = idx_p.tile([128, 16], F32, tag=f"vh{r}")
            nc.sync.dma_start(vh[:], bd_f[:, h * 2048 : (h + 1) * 2048])
            pw = ps_tile([16, 128])
            nc.tensor.matmul(out=pw[:], lhsT=vh[:], rhs=ident[:], start=True, stop=True)
            nc.vector.tensor_copy(w_sb[:, h * 128 : (h + 1) * 128], pw[:])
        # column sums -> exclusive prefix along q
        pcs = ps_tile([1, 256])
        nc.tensor.matmul(out=pcs[:], lhsT=ones_col16[:], rhs=w_sb[:], start=True, stop=True)
        cs_sb = idx_p.tile([1, 256], F32, tag=f"cs{r}")
        nc.vector.tensor_copy(cs_sb[:], pcs[:])
        incl = idx_p.tile([1, 256], F32, tag=f"incl{r}")
        nc.vector.tensor_tensor_scan(
            out=incl[:], data0=cs_sb[:], data1=zeros_row[:],
            initial=0.0, op0=OP.add, op1=OP.add,
        )
        excl = idx_p.tile([1, 256], F32, tag=f"excl{r}")
        nc.vector.tensor_tensor(out=excl[:], in0=incl[:], in1=cs_sb[:], op=OP.subtract)
        # full cumsum = tri16 @ W + broadcast(excl)
        pidx = ps_tile([16, 256])
        nc.tensor.matmul(out=pidx[:], lhsT=tri16[:], rhs=w_sb[:], start=True, stop=False)
        nc.tensor.matmul(
            out=pidx[:], lhsT=ones_row[:, :16], rhs=excl[:], start=False, stop=True
        )
        idxf = idx_p.tile([16, 256], F32, tag=f"idxf{r}")
        nc.vector.tensor_scalar(
            out=idxf[:], in0=pidx[:], scalar1=-1.0, scalar2=0.0, op0=OP.add, op1=OP.max
        )
        nc.vector.tensor_scalar_min(idxf[:], idxf[:], float(J - 1))
        idx16 = idx_p.tile([16, 256], I16, tag=f"idx16{r}")
        nc.vector.tensor_copy(idx16[:], idxf[:])
        rep = idx_p.tile([128, 256], I16, tag=f"rep{r}")
        for k in range(8):
            nc.sync.dma_start(rep[k * 16 : (k + 1) * 16, :], idx16[:])
        idx_rep.append(rep)

    # --- phase 3: EMA ---
    # eTall[r] column layout: [d*J + j] — D-block-major, j within block.
    etT = {}
    for r in range(BL):
        etT[r] = etT_p.tile([128, NDB * J], F32, tag="etT", name=f"etT{r}")
    for r in range(BL):
        for g in range(NCH):
            e_t = et_p.tile([T, D], F32, tag="et", name=f"et{r}_{g}")
            nc.sync.dma_start(e_t[:], emb[r, g * T : (g + 1) * T, :])
            pt = ps_tile([128, D])
            for d in range(NDB):
                # (e_block)^T @ diag(c) : transpose + per-column scale by c_j
                nc.tensor.matmul(
                    out=pt[:, d * 128 : (d + 1) * 128],
                    lhsT=e_t[:, d * 128 : (d + 1) * 128],
                    rhs=diag_c[(r, g)][:], start=True, stop=True,
                )
            dst = etT[r][:].rearrange("p (d j) -> p d j", d=NDB)[
                :, :, g * T : (g + 1) * T
            ]
            src = pt[:].rearrange("p (d j) -> p d j", d=NDB)
            if g % 2 == 0:
                nc.vector.tensor_copy(dst, src)
            else:
                nc.scalar.copy(dst, src)

    smT = {}
    for r in range(BL):
        for d in range(NDB):
            st = smT_p.tile([128, J], F32, tag="smT", name=f"smT{r}_{d}")
            nc.vector.tensor_tensor_scan(
                out=st[:], data0=a_bc[r][:],
                data1=etT[r][:, d * J : (d + 1) * J],
                initial=0.0, op0=OP.mult, op1=OP.add,
            )
            smT[(r, d)] = st

    for r in range(BL):
        for g in range(NCH):
            smn = smn_p.tile([T, D], F32, tag="smn", name=f"smn{r}_{g}")
            pt2 = ps_tile([128, D])
            for d in range(NDB):
                nc.tensor.matmul(
                    out=pt2[:, d * 128 : (d + 1) * 128],
                    lhsT=smT[(r, d)][:, g * T : (g + 1) * T],
                    rhs=ident[:], start=True, stop=True, is_transpose=True,
                )
            if g % 2 == 0:
                nc.vector.tensor_copy(smn[:], pt2[:])
            else:
                nc.scalar.copy(smn[:], pt2[:])
            nc.scalar.dma_start(smoothed[r][g * T : (g + 1) * T, :], smn[:])

    # --- phase 4: gather + store ---
    for r in range(BL):
        for s in range(NSUB):
            gt = gout_p.tile([128, SUBL // 128, D], F32, tag="gout", name=f"gout{r}_{s}")
            nc.gpsimd.dma_gather(
                out_ap=gt[:],
                in_ap=smoothed[r][:],
                idxs_ap=idx_rep[r][:, s * (SUBL // 16) : (s + 1) * (SUBL // 16)],
                num_idxs=SUBL,
                num_idxs_reg=SUBL,
                elem_size=D,
                queue_num=(r * NSUB + s) % 4,
            )
            dst = out[r, s * SUBL : (s + 1) * SUBL, :].rearrange(
                "(g p) d -> p g d", p=128
            )
            nc.sync.dma_start(dst, gt[:])


def _patch_swdge_lane_by_queue():
    """Tile assigns DMASW completion-sem lanes round-robin, queue-blind; the
    HW/sim lock each lane's sem to one SWDGE queue. Pin lane = queue_num so
    multi-queue gathers get consistent lanes."""
    from concourse import bass_isa
    from concourse import tile_sem_assignment as tsa

    if getattr(tsa.TileClockTick, "_ema_queue_patch", False):
        return
    orig = tsa.TileClockTick._assign_tick

    def patched(self, inst):
        if (
            isinstance(inst, bass_isa.AnyDMAInstruction)
            and inst.engine == mybir.EngineType.Pool
            and not isinstance(inst, bass_isa.UserSyncedRemoteDMADescs)
        ):
            self.next_sw_dma_idx = getattr(inst, "queue_num", 0) or 0
        return orig(self, inst)

    tsa.TileClockTick._assign_tick = patched
    tsa.TileClockTick._ema_queue_patch = True


def build():
    _patch_swdge_lane_by_queue()
    nc = bacc.Bacc(
        "TRN2",
        target_bir_lowering=False,
        debug=False,
        enable_asserts=False,
        num_devices=N_CORES,
        num_swdge_queues=4,
    )
    with tile.TileContext(nc) as tc, ExitStack() as ctx:
        _body(tc, ctx)
    nc.compile()
    return nc


def make_in_maps(inputs):
    emb = np.asarray(inputs["unit_embeddings"], dtype=np.float32)
    conf = np.asarray(inputs["unit_confidence"], dtype=np.float32)
    msk = np.asarray(inputs["unit_mask"]).astype(np.uint8)
    bd = np.asarray(inputs["boundary_mask"]).astype(np.uint8)
    in_maps = []
    for c in range(N_CORES):
        sl = slice(c * BL, (c + 1) * BL)
        in_maps.append(
            {
                "unit_embeddings": np.ascontiguousarray(emb[sl]),
                "unit_confidence": np.ascontiguousarray(conf[sl]),
                "unit_mask": np.ascontiguousarray(msk[sl]),
                "boundary_mask": np.ascontiguousarray(bd[sl]),
            }
        )
    return in_maps


_cached_nc = None


def run(inputs, trace=False):
    global _cached_nc
    if _cached_nc is None:
        _cached_nc = build()
    res = run_bass_kernel_spmd(
        _cached_nc, make_in_maps(inputs), core_ids=list(range(N_CORES)), trace=trace
    )
    full = np.concatenate(
        [res.results[c]["frames"] for c in range(N_CORES)], axis=0
    )
    return full, res


def kernel(**inputs) -> np.ndarray:
    full, _ = run(inputs, trace=False)
    return full


# revision 15
# speedup vs baseline: 1.0976x; 1.0976x over previous
"""EMA dechunker kernel for Trainium2 (Bass/Tile), 8-core data-parallel.

Problem: for each batch row
  smoothed[j] = m[j] ? clip(p[j])*emb[j] + (1-clip(p[j]))*smoothed[j-1]
                     : smoothed[j-1]
  frames[l]   = smoothed[clip(cumsum(boundary)[l]-1, 0, J-1)]

Sharding: batch dim B=16 split across 8 cores (2 rows/core). Each core:
  1. coeffs: c = clip(conf)*mask, a = 1-c  (tiny row ops)
  2. EMA: PE-transpose emb blocks (scaled by diag(c)) into (D-part, J-free)
     layout, then one tensor_tensor_scan per (row, D-block) runs the whole
     first-order recurrence along the free dim. Transpose back, store
     smoothed to DRAM.
  3. idx: two-level cumsum of boundary mask (PE tri-matmul over 16
     partitions + free-dim scan of column sums), -1, clip, cast int16.
  4. gather: dma_gather pulls frames' source rows from DRAM smoothed,
     direct DMA writes the output.
"""

from contextlib import ExitStack

import numpy as np

import concourse.bass as bass
import concourse.tile as tile
from concourse import bacc, mybir
from concourse.bass_utils import run_bass_kernel_spmd
from concourse.masks import make_identity

F32 = mybir.dt.float32
I16 = mybir.dt.int16
U8 = mybir.dt.uint8
OP = mybir.AluOpType

B, J, L, D = 16, 1024, 4096, 512
N_CORES = 8
BL = B // N_CORES          # 2 batch rows per core
T = 128                    # j-chunk (partition) size
NCH = J // T               # 8 chunks per row
NDB = D // 128             # 4 D-blocks of 128 partitions
NSUB = 4                   # sub-gathers per row
SUBL = L // NSUB           # 1024 frames per sub-gather
EPS = 1e-4


def _body(tc, ctx):
    nc = tc.nc
    emb = nc.dram_tensor("unit_embeddings", [BL, J, D], F32, kind="ExternalInput").ap()
    conf = nc.dram_tensor("unit_confidence", [BL, J], F32, kind="ExternalInput").ap()
    mask = nc.dram_tensor("unit_mask", [BL, J], U8, kind="ExternalInput").ap()
    bdry = nc.dram_tensor("boundary_mask", [BL, L], U8, kind="ExternalInput").ap()
    out = nc.dram_tensor("frames", [BL, L, D], F32, kind="ExternalOutput").ap()

    const_p = ctx.enter_context(tc.tile_pool(name="const", bufs=1))
    coef_p = ctx.enter_context(tc.tile_pool(name="coef", bufs=1))
    et_p = ctx.enter_context(tc.tile_pool(name="et", bufs=3))
    etT_p = ctx.enter_context(tc.tile_pool(name="etT", bufs=BL))
    smT_p = ctx.enter_context(tc.tile_pool(name="smT", bufs=2 * NDB))
    smn_p = ctx.enter_context(tc.tile_pool(name="smn", bufs=3))
    diag_p = ctx.enter_context(tc.tile_pool(name="diag", bufs=4))
    idx_p = ctx.enter_context(tc.tile_pool(name="idx", bufs=1))
    gout_p = ctx.enter_context(tc.tile_pool(name="gout", bufs=2))
    dram_p = ctx.enter_context(tc.tile_pool(name="dram", bufs=1, space="DRAM"))
    psum_p = ctx.enter_context(tc.tile_pool(name="psum", bufs=7, space="PSUM"))

    ps_ctr = [0]

    def ps_tile(shape):
        ps_ctr[0] += 1
        return psum_p.tile(shape, F32, tag="ps", name=f"ps{ps_ctr[0]}")

    # --- constants ---
    ident = const_p.tile([128, 128], F32)
    make_identity(nc, ident[:])
    ones_row = const_p.tile([1, 128], F32)
    nc.gpsimd.memset(ones_row[:], 1.0)
    ones_col16 = const_p.tile([16, 1], F32)
    nc.gpsimd.memset(ones_col16[:], 1.0)
    zeros_row = const_p.tile([1, 256], F32)
    nc.gpsimd.memset(zeros_row[:], 0.0)
    # tri16[k, p] = 1 iff k <= p  (lhsT for partition-dim inclusive cumsum):
    # running-sum of the identity along the free dim.
    zeros16 = const_p.tile([16, 16], F32)
    nc.gpsimd.memset(zeros16[:], 0.0)
    tri16 = const_p.tile([16, 16], F32)
    nc.vector.tensor_tensor_scan(
        out=tri16[:], data0=zeros16[:], data1=ident[:16, :16],
        initial=0.0, op0=OP.add, op1=OP.add,
    )

    smoothed = [dram_p.tile([J, D], F32, name=f"smoothed{r}") for r in range(BL)]

    # --- phase 1: coefficients ---
    c_rows = []
    a_bc = []
    for r in range(BL):
        cf = coef_p.tile([1, J], F32, tag=f"cf{r}")
        nc.sync.dma_start(cf[:], conf[r : r + 1, :])
        mk = coef_p.tile([1, J], F32, tag=f"mk{r}")
        nc.gpsimd.dma_start(mk[:], mask[r : r + 1, :])  # u8 -> f32 cast in DMA
        c_r = coef_p.tile([1, J], F32, tag=f"c{r}")
        nc.vector.tensor_scalar(
            out=c_r[:], in0=cf[:], scalar1=EPS, scalar2=1.0 - EPS,
            op0=OP.max, op1=OP.min,
        )
        nc.vector.tensor_tensor(out=c_r[:], in0=c_r[:], in1=mk[:], op=OP.mult)
        a_r = coef_p.tile([1, J], F32, tag=f"a{r}")
        nc.vector.tensor_scalar(
            out=a_r[:], in0=c_r[:], scalar1=-1.0, scalar2=1.0,
            op0=OP.mult, op1=OP.add,
        )
        c_rows.append(c_r)
        # broadcast a to 128 partitions via K=1 matmul
        abc = coef_p.tile([128, J], F32, tag=f"abc{r}")
        for h in range(J // 512):
            pb = ps_tile([128, 512])
            nc.tensor.matmul(
                out=pb[:], lhsT=ones_row[:], rhs=a_r[:, h * 512 : (h + 1) * 512],
                start=True, stop=True,
            )
            nc.scalar.copy(abc[:, h * 512 : (h + 1) * 512], pb[:])
        a_bc.append(abc)

    # c columns: cstage[(r*8+g), :] = c_r[g*128:(g+1)*128] ; transpose -> (128, 16)
    cstage = coef_p.tile([2 * NCH, T], F32)
    for r in range(BL):
        nc.sync.dma_start(cstage[r * NCH : (r + 1) * NCH, :], c_rows[r][:])
    pc = ps_tile([128, 2 * NCH])
    nc.tensor.matmul(
        out=pc[:], lhsT=cstage[:], rhs=ident[: 2 * NCH, : 2 * NCH],
        start=True, stop=True,
    )
    c_cols = coef_p.tile([128, 2 * NCH], F32)
    nc.vector.tensor_copy(c_cols[:], pc[:])



    # --- phase 2: indices ---
    idx_rep = []
    for r in range(BL):
        bd_f = idx_p.tile([1, L], F32, tag=f"bdf{r}")
        nc.gpsimd.dma_start(bd_f[:], bdry[r : r + 1, :])  # u8 -> f32
        # W[p, q] = bd[q*16 + p] for p in [0,16), q in [0,256)
        w_sb = idx_p.tile([16, 256], F32, tag=f"w{r}")
        for h in range(2):
            vh = idx_p.tile([128, 16], F32, tag=f"vh{r}")
            nc.sync.dma_start(vh[:], bd_f[:, h * 2048 : (h + 1) * 2048])
            pw = ps_tile([16, 128])
            nc.tensor.matmul(out=pw[:], lhsT=vh[:], rhs=ident[:], start=True, stop=True)
            nc.vector.tensor_copy(w_sb[:, h * 128 : (h + 1) * 128], pw[:])
        # column sums -> exclusive prefix along q
        pcs = ps_tile([1, 256])
        nc.tensor.matmul(out=pcs[:], lhsT=ones_col16[:], rhs=w_sb[:], start=True, stop=True)
        cs_sb = idx_p.tile([1, 256], F32, tag=f"cs{r}")
        nc.vector.tensor_copy(cs_sb[:], pcs[:])
        incl = idx_p.tile([1, 256], F32, tag=f"incl{r}")
        nc.vector.tensor_tensor_scan(
            out=incl[:], data0=cs_sb[:], data1=zeros_row[:],
            initial=0.0, op0=OP.add, op1=OP.add,
        )
        excl = idx_p.tile([1, 256], F32, tag=f"excl{r}")
        nc.vector.tensor_tensor(out=excl[:], in0=incl[:], in1=cs_sb[:], op=OP.subtract)
        # full cumsum = tri16 @ W + broadcast(excl)
        pidx = ps_tile([16, 256])
        nc.tensor.matmul(out=pidx[:], lhsT=tri16[:], rhs=w_sb[:], start=True, stop=False)
        nc.tensor.matmul(
            out=pidx[:], lhsT=ones_row[:, :16], rhs=excl[:], start=False, stop=True
        )
        idxf = idx_p.tile([16, 256], F32, tag=f"idxf{r}")
        nc.vector.tensor_scalar(
            out=idxf[:], in0=pidx[:], scalar1=-1.0, scalar2=0.0, op0=OP.add, op1=OP.max
        )
        nc.vector.tensor_scalar_min(idxf[:], idxf[:], float(J - 1))
        idx16 = idx_p.tile([16, 256], I16, tag=f"idx16{r}")
        nc.vector.tensor_copy(idx16[:], idxf[:])
        rep = idx_p.tile([128, 256], I16, tag=f"rep{r}")
        for k in range(8):
            nc.sync.dma_start(rep[k * 16 : (k + 1) * 16, :], idx16[:])
        idx_rep.append(rep)

    # --- phases 3+4, pipelined per batch row ---
    # eTall[r] column layout: [d*J + j] — D-block-major, j within block.
    etT = {}
    for r in range(BL):
        etT[r] = etT_p.tile([128, NDB * J], F32, tag="etT", name=f"etT{r}")

    def ema_row(r):
        for g in range(NCH):
            e_t = et_p.tile([T, D], F32, tag="et", name=f"et{r}_{g}")
            nc.sync.dma_start(e_t[:], emb[r, g * T : (g + 1) * T, :])
            # scale rows by c (per-partition scalar broadcast along D)
            col = r * NCH + g
            nc.vector.tensor_tensor(
                out=e_t[:], in0=e_t[:],
                in1=c_cols[:, col : col + 1].to_broadcast([T, D]), op=OP.mult,
            )
            pt = ps_tile([128, D])
            for d in range(NDB):
                nc.tensor.matmul(
                    out=pt[:, d * 128 : (d + 1) * 128],
                    lhsT=e_t[:, d * 128 : (d + 1) * 128],
                    rhs=ident[:], start=True, stop=True,
                    is_transpose=True,
                )
            dst = etT[r][:].rearrange("p (d j) -> p d j", d=NDB)[
                :, :, g * T : (g + 1) * T
            ]
            src = pt[:].rearrange("p (d j) -> p d j", d=NDB)
            if g % 2 == 0:
                nc.vector.tensor_copy(dst, src)
            else:
                nc.scalar.copy(dst, src)

        smT = {}
        for d in range(NDB):
            st = smT_p.tile([128, J], F32, tag="smT", name=f"smT{r}_{d}")
            nc.vector.tensor_tensor_scan(
                out=st[:], data0=a_bc[r][:],
                data1=etT[r][:, d * J : (d + 1) * J],
                initial=0.0, op0=OP.mult, op1=OP.add,
            )
            smT[d] = st

        for g in range(NCH):
            smn = smn_p.tile([T, D], F32, tag="smn", name=f"smn{r}_{g}")
            pt2 = ps_tile([128, D])
            for d in range(NDB):
                nc.tensor.matmul(
                    out=pt2[:, d * 128 : (d + 1) * 128],
                    lhsT=smT[d][:, g * T : (g + 1) * T],
                    rhs=ident[:], start=True, stop=True, is_transpose=True,
                )
            if g % 2 == 0:
                nc.vector.tensor_copy(smn[:], pt2[:])
            else:
                nc.scalar.copy(smn[:], pt2[:])
            nc.scalar.dma_start(smoothed[r][g * T : (g + 1) * T, :], smn[:])

    def gather_row(r):
        for s in range(NSUB):
            gt = gout_p.tile([128, SUBL // 128, D], F32, tag="gout", name=f"gout{r}_{s}")
            nc.gpsimd.dma_gather(
                out_ap=gt[:],
                in_ap=smoothed[r][:],
                idxs_ap=idx_rep[r][:, s * (SUBL // 16) : (s + 1) * (SUBL // 16)],
                num_idxs=SUBL,
                num_idxs_reg=SUBL,
                elem_size=D,
                queue_num=0,
            )
            dst = out[r, s * SUBL : (s + 1) * SUBL, :].rearrange(
                "(g p) d -> p g d", p=128
            )
            nc.sync.dma_start(dst, gt[:])

    ema_row(0)
    gather_row(0)
    ema_row(1)
    gather_row(1)


def _patch_swdge_lane_by_queue():
    """Tile assigns DMASW completion-sem lanes round-robin, queue-blind; the
    HW/sim lock each lane's sem to one SWDGE queue. Pin lane = queue_num so
    multi-queue gathers get consistent lanes."""
    from concourse import bass_isa
    from concourse import tile_sem_assignment as tsa

    if getattr(tsa.TileClockTick, "_ema_queue_patch", False):
        return
    orig = tsa.TileClockTick._assign_tick

    def patched(self, inst):
        if (
            isinstance(inst, bass_isa.AnyDMAInstruction)
            and inst.engine == mybir.EngineType.Pool
            and not isinstance(inst, bass_isa.UserSyncedRemoteDMADescs)
        ):
            self.next_sw_dma_idx = getattr(inst, "queue_num", 0) or 0
        return orig(self, inst)

    tsa.TileClockTick._assign_tick = patched
    tsa.TileClockTick._ema_queue_patch = True


def build():
    _patch_swdge_lane_by_queue()
    nc = bacc.Bacc(
        "TRN2",
        target_bir_lowering=False,
        debug=False,
        enable_asserts=False,
        num_devices=N_CORES,
        num_swdge_queues=1,
    )
    with tile.TileContext(nc) as tc, ExitStack() as ctx:
        _body(tc, ctx)
    nc.compile()
    return nc


def make_in_maps(inputs):
    emb = np.asarray(inputs["unit_embeddings"], dtype=np.float32)
    conf = np.asarray(inputs["unit_confidence"], dtype=np.float32)
    msk = np.asarray(inputs["unit_mask"]).astype(np.uint8)
    bd = np.asarray(inputs["boundary_mask"]).astype(np.uint8)
    in_maps = []
    for c in range(N_CORES):
        sl = slice(c * BL, (c + 1) * BL)
        in_maps.append(
            {
                "unit_embeddings": np.ascontiguousarray(emb[sl]),
                "unit_confidence": np.ascontiguousarray(conf[sl]),
                "unit_mask": np.ascontiguousarray(msk[sl]),
                "boundary_mask": np.ascontiguousarray(bd[sl]),
            }
        )
    return in_maps


_cached_nc = None


def run(inputs, trace=False):
    global _cached_nc
    if _cached_nc is None:
        _cached_nc = build()
    res = run_bass_kernel_spmd(
        _cached_nc, make_in_maps(inputs), core_ids=list(range(N_CORES)), trace=trace
    )
    full = np.concatenate(
        [res.results[c]["frames"] for c in range(N_CORES)], axis=0
    )
    return full, res


def kernel(**inputs) -> np.ndarray:
    full, _ = run(inputs, trace=False)
    return full


# revision 17
# speedup vs baseline: 1.1319x; 1.0313x over previous
"""EMA dechunker kernel for Trainium2 (Bass/Tile), 8-core data-parallel.

Problem: for each batch row
  smoothed[j] = m[j] ? clip(p[j])*emb[j] + (1-clip(p[j]))*smoothed[j-1]
                     : smoothed[j-1]
  frames[l]   = smoothed[clip(cumsum(boundary)[l]-1, 0, J-1)]

Sharding: batch dim B=16 split across 8 cores (2 rows/core). Each core:
  1. coeffs: c = clip(conf)*mask, a = 1-c  (tiny row ops)
  2. EMA: PE-transpose emb blocks (scaled by diag(c)) into (D-part, J-free)
     layout, then one tensor_tensor_scan per (row, D-block) runs the whole
     first-order recurrence along the free dim. Transpose back, store
     smoothed to DRAM.
  3. idx: two-level cumsum of boundary mask (PE tri-matmul over 16
     partitions + free-dim scan of column sums), -1, clip, cast int16.
  4. gather: dma_gather pulls frames' source rows from DRAM smoothed,
     direct DMA writes the output.
"""

from contextlib import ExitStack

import numpy as np

import concourse.bass as bass
import concourse.tile as tile
from concourse import bacc, mybir
from concourse.bass_utils import run_bass_kernel_spmd
from concourse.masks import make_identity

F32 = mybir.dt.float32
I16 = mybir.dt.int16
U8 = mybir.dt.uint8
OP = mybir.AluOpType

B, J, L, D = 16, 1024, 4096, 512
N_CORES = 8
BL = B // N_CORES          # 2 batch rows per core
T = 128                    # j-chunk (partition) size
NCH = J // T               # 8 chunks per row
NDB = D // 128             # 4 D-blocks of 128 partitions
NSUB = 4                   # sub-gathers per row
SUBL = L // NSUB           # 1024 frames per sub-gather
EPS = 1e-4


def _body(tc, ctx):
    nc = tc.nc
    emb = nc.dram_tensor("unit_embeddings", [BL, J, D], F32, kind="ExternalInput").ap()
    conf = nc.dram_tensor("unit_confidence", [BL, J], F32, kind="ExternalInput").ap()
    mask = nc.dram_tensor("unit_mask", [BL, J], U8, kind="ExternalInput").ap()
    bdry = nc.dram_tensor("boundary_mask", [BL, L], U8, kind="ExternalInput").ap()
    out = nc.dram_tensor("frames", [BL, L, D], F32, kind="ExternalOutput").ap()

    const_p = ctx.enter_context(tc.tile_pool(name="const", bufs=1))
    coef_p = ctx.enter_context(tc.tile_pool(name="coef", bufs=1))
    et_p = ctx.enter_context(tc.tile_pool(name="et", bufs=3))
    etT_p = ctx.enter_context(tc.tile_pool(name="etT", bufs=BL))
    smT_p = ctx.enter_context(tc.tile_pool(name="smT", bufs=2 * NDB))
    smn_p = ctx.enter_context(tc.tile_pool(name="smn", bufs=3))
    diag_p = ctx.enter_context(tc.tile_pool(name="diag", bufs=4))
    idx_p = ctx.enter_context(tc.tile_pool(name="idx", bufs=1))
    gout_p = ctx.enter_context(tc.tile_pool(name="gout", bufs=2))
    dram_p = ctx.enter_context(tc.tile_pool(name="dram", bufs=1, space="DRAM"))
    psum_p = ctx.enter_context(tc.tile_pool(name="psum", bufs=7, space="PSUM"))

    ps_ctr = [0]

    def ps_tile(shape):
        ps_ctr[0] += 1
        return psum_p.tile(shape, F32, tag="ps", name=f"ps{ps_ctr[0]}")

    # --- constants ---
    ident = const_p.tile([128, 128], F32)
    make_identity(nc, ident[:])
    ones_row = const_p.tile([1, 128], F32)
    nc.gpsimd.memset(ones_row[:], 1.0)
    ones_col16 = const_p.tile([16, 1], F32)
    nc.gpsimd.memset(ones_col16[:], 1.0)
    zeros_row = const_p.tile([1, 256], F32)
    nc.gpsimd.memset(zeros_row[:], 0.0)
    # tri16[k, p] = 1 iff k <= p  (lhsT for partition-dim inclusive cumsum):
    # running-sum of the identity along the free dim.
    zeros16 = const_p.tile([16, 16], F32)
    nc.gpsimd.memset(zeros16[:], 0.0)
    tri16 = const_p.tile([16, 16], F32)
    nc.vector.tensor_tensor_scan(
        out=tri16[:], data0=zeros16[:], data1=ident[:16, :16],
        initial=0.0, op0=OP.add, op1=OP.add,
    )

    smoothed = [dram_p.tile([J, D], F32, name=f"smoothed{r}") for r in range(BL)]

    # --- phase 1: coefficients ---
    c_rows = []
    a_bc = []
    for r in range(BL):
        cf = coef_p.tile([1, J], F32, tag=f"cf{r}")
        nc.sync.dma_start(cf[:], conf[r : r + 1, :])
        mk = coef_p.tile([1, J], F32, tag=f"mk{r}")
        nc.gpsimd.dma_start(mk[:], mask[r : r + 1, :])  # u8 -> f32 cast in DMA
        c_r = coef_p.tile([1, J], F32, tag=f"c{r}")
        nc.vector.tensor_scalar(
            out=c_r[:], in0=cf[:], scalar1=EPS, scalar2=1.0 - EPS,
            op0=OP.max, op1=OP.min,
        )
        nc.vector.tensor_tensor(out=c_r[:], in0=c_r[:], in1=mk[:], op=OP.mult)
        a_r = coef_p.tile([1, J], F32, tag=f"a{r}")
        nc.vector.tensor_scalar(
            out=a_r[:], in0=c_r[:], scalar1=-1.0, scalar2=1.0,
            op0=OP.mult, op1=OP.add,
        )
        c_rows.append(c_r)
        # broadcast a to 128 partitions via K=1 matmul
        abc = coef_p.tile([128, J], F32, tag=f"abc{r}")
        for h in range(J // 512):
            pb = ps_tile([128, 512])
            nc.tensor.matmul(
                out=pb[:], lhsT=ones_row[:], rhs=a_r[:, h * 512 : (h + 1) * 512],
                start=True, stop=True,
            )
            nc.scalar.copy(abc[:, h * 512 : (h + 1) * 512], pb[:])
        a_bc.append(abc)

    # c columns: cstage[(r*8+g), :] = c_r[g*128:(g+1)*128] ; transpose -> (128, 16)
    cstage = coef_p.tile([2 * NCH, T], F32)
    for r in range(BL):
        nc.sync.dma_start(cstage[r * NCH : (r + 1) * NCH, :], c_rows[r][:])
    pc = ps_tile([128, 2 * NCH])
    nc.tensor.matmul(
        out=pc[:], lhsT=cstage[:], rhs=ident[: 2 * NCH, : 2 * NCH],
        start=True, stop=True,
    )
    c_cols = coef_p.tile([128, 2 * NCH], F32)
    nc.vector.tensor_copy(c_cols[:], pc[:])



    # --- phase 2: indices ---
    idx_rep = []
    for r in range(BL):
        bd_f = idx_p.tile([1, L], F32, tag=f"bdf{r}")
        nc.gpsimd.dma_start(bd_f[:], bdry[r : r + 1, :])  # u8 -> f32
        # W[p, q] = bd[q*16 + p] for p in [0,16), q in [0,256)
        w_sb = idx_p.tile([16, 256], F32, tag=f"w{r}")
        for h in range(2):
            vh = idx_p.tile([128, 16], F32, tag=f"vh{r}")
            nc.sync.dma_start(vh[:], bd_f[:, h * 2048 : (h + 1) * 2048])
            pw = ps_tile([16, 128])
            nc.tensor.matmul(out=pw[:], lhsT=vh[:], rhs=ident[:], start=True, stop=True)
            nc.vector.tensor_copy(w_sb[:, h * 128 : (h + 1) * 128], pw[:])
        # column sums -> exclusive prefix along q
        pcs = ps_tile([1, 256])
        nc.tensor.matmul(out=pcs[:], lhsT=ones_col16[:], rhs=w_sb[:], start=True, stop=True)
        cs_sb = idx_p.tile([1, 256], F32, tag=f"cs{r}")
        nc.vector.tensor_copy(cs_sb[:], pcs[:])
        incl = idx_p.tile([1, 256], F32, tag=f"incl{r}")
        nc.vector.tensor_tensor_scan(
            out=incl[:], data0=cs_sb[:], data1=zeros_row[:],
            initial=0.0, op0=OP.add, op1=OP.add,
        )
        excl = idx_p.tile([1, 256], F32, tag=f"excl{r}")
        nc.vector.tensor_tensor(out=excl[:], in0=incl[:], in1=cs_sb[:], op=OP.subtract)
        # full cumsum = tri16 @ W + broadcast(excl)
        pidx = ps_tile([16, 256])
        nc.tensor.matmul(out=pidx[:], lhsT=tri16[:], rhs=w_sb[:], start=True, stop=False)
        nc.tensor.matmul(
            out=pidx[:], lhsT=ones_row[:, :16], rhs=excl[:], start=False, stop=True
        )
        idxf = idx_p.tile([16, 256], F32, tag=f"idxf{r}")
        nc.vector.tensor_scalar(
            out=idxf[:], in0=pidx[:], scalar1=-1.0, scalar2=0.0, op0=OP.add, op1=OP.max
        )
        nc.vector.tensor_scalar_min(idxf[:], idxf[:], float(J - 1))
        idx16 = idx_p.tile([16, 256], I16, tag=f"idx16{r}")
        nc.vector.tensor_copy(idx16[:], idxf[:])
        rep = idx_p.tile([128, 256], I16, tag=f"rep{r}")
        for k in range(8):
            nc.sync.dma_start(rep[k * 16 : (k + 1) * 16, :], idx16[:])
        idx_rep.append(rep)

    # --- phases 3+4, pipelined per batch row ---
    # eTall[r] column layout: [d*J + j] — D-block-major, j within block.
    etT = {}
    for r in range(BL):
        etT[r] = etT_p.tile([128, NDB * J], F32, tag="etT", name=f"etT{r}")

    def ema_row(r):
        for g in range(NCH):
            e_t = et_p.tile([T, D], F32, tag="et", name=f"et{r}_{g}")
            nc.sync.dma_start(e_t[:], emb[r, g * T : (g + 1) * T, :])
            # scale rows by c (per-partition scalar broadcast along D)
            col = r * NCH + g
            nc.vector.tensor_tensor(
                out=e_t[:], in0=e_t[:],
                in1=c_cols[:, col : col + 1].to_broadcast([T, D]), op=OP.mult,
            )
            pt = ps_tile([128, D])
            for d in range(NDB):
                nc.tensor.matmul(
                    out=pt[:, d * 128 : (d + 1) * 128],
                    lhsT=e_t[:, d * 128 : (d + 1) * 128],
                    rhs=ident[:], start=True, stop=True,
                    is_transpose=True,
                )
            dst = etT[r][:].rearrange("p (d j) -> p d j", d=NDB)[
                :, :, g * T : (g + 1) * T
            ]
            src = pt[:].rearrange("p (d j) -> p d j", d=NDB)
            if g % 2 == 0:
                nc.vector.tensor_copy(dst, src)
            else:
                nc.scalar.copy(dst, src)

        smT = {}
        for d in range(NDB):
            st = smT_p.tile([128, J], F32, tag="smT", name=f"smT{r}_{d}")
            nc.vector.tensor_tensor_scan(
                out=st[:], data0=a_bc[r][:],
                data1=etT[r][:, d * J : (d + 1) * J],
                initial=0.0, op0=OP.mult, op1=OP.add,
            )
            smT[d] = st

        for g in range(NCH):
            smn = smn_p.tile([T, D], F32, tag="smn", name=f"smn{r}_{g}")
            pt2 = ps_tile([128, D])
            for d in range(NDB):
                nc.tensor.matmul(
                    out=pt2[:, d * 128 : (d + 1) * 128],
                    lhsT=smT[d][:, g * T : (g + 1) * T],
                    rhs=ident[:], start=True, stop=True, is_transpose=True,
                )
            if g % 2 == 0:
                nc.vector.tensor_copy(smn[:], pt2[:])
            else:
                nc.scalar.copy(smn[:], pt2[:])
            nc.scalar.dma_start(smoothed[r][g * T : (g + 1) * T, :], smn[:])

    def gather_row(r):
        for s in range(NSUB):
            gt = gout_p.tile([128, SUBL // 128, D], F32, tag="gout", name=f"gout{r}_{s}")
            nc.gpsimd.dma_gather(
                out_ap=gt[:],
                in_ap=smoothed[r][:],
                idxs_ap=idx_rep[r][:, s * (SUBL // 16) : (s + 1) * (SUBL // 16)],
                num_idxs=SUBL,
                num_idxs_reg=SUBL,
                elem_size=D,
                queue_num=s % 2,
            )
            dst = out[r, s * SUBL : (s + 1) * SUBL, :].rearrange(
                "(g p) d -> p g d", p=128
            )
            if s % 2 == 0:
                nc.sync.dma_start(dst, gt[:])
            else:
                nc.scalar.dma_start(dst, gt[:])

    ema_row(0)
    gather_row(0)
    ema_row(1)
    gather_row(1)


def _patch_swdge_lane_by_queue():
    """Tile assigns DMASW completion-sem lanes round-robin, queue-blind; the
    HW/sim lock each lane's sem to one SWDGE queue. Pin lane = queue_num so
    multi-queue gathers get consistent lanes."""
    from concourse import bass_isa
    from concourse import tile_sem_assignment as tsa

    if getattr(tsa.TileClockTick, "_ema_queue_patch", False):
        return
    orig = tsa.TileClockTick._assign_tick

    def patched(self, inst):
        if (
            isinstance(inst, bass_isa.AnyDMAInstruction)
            and inst.engine == mybir.EngineType.Pool
            and not isinstance(inst, bass_isa.UserSyncedRemoteDMADescs)
        ):
            self.next_sw_dma_idx = getattr(inst, "queue_num", 0) or 0
        return orig(self, inst)

    tsa.TileClockTick._assign_tick = patched
    tsa.TileClockTick._ema_queue_patch = True


def build():
    _patch_swdge_lane_by_queue()
    nc = bacc.Bacc(
        "TRN2",
        target_bir_lowering=False,
        debug=False,
        enable_asserts=False,
        num_devices=N_CORES,
        num_swdge_queues=2,
        dynamic_dma_scratch_size=28672,
    )
    with tile.TileContext(nc) as tc, ExitStack() as ctx:
        _body(tc, ctx)
    nc.compile()
    return nc


def make_in_maps(inputs):
    emb = np.asarray(inputs["unit_embeddings"], dtype=np.float32)
    conf = np.asarray(inputs["unit_confidence"], dtype=np.float32)
    msk = np.asarray(inputs["unit_mask"]).astype(np.uint8)
    bd = np.asarray(inputs["boundary_mask"]).astype(np.uint8)
    in_maps = []
    for c in range(N_CORES):
        sl = slice(c * BL, (c + 1) * BL)
        in_maps.append(
            {
                "unit_embeddings": np.ascontiguousarray(emb[sl]),
                "unit_confidence": np.ascontiguousarray(conf[sl]),
                "unit_mask": np.ascontiguousarray(msk[sl]),
                "boundary_mask": np.ascontiguousarray(bd[sl]),
            }
        )
    return in_maps


_cached_nc = None


def run(inputs, trace=False):
    global _cached_nc
    if _cached_nc is None:
        _cached_nc = build()
    res = run_bass_kernel_spmd(
        _cached_nc, make_in_maps(inputs), core_ids=list(range(N_CORES)), trace=trace
    )
    full = np.concatenate(
        [res.results[c]["frames"] for c in range(N_CORES)], axis=0
    )
    return full, res


def kernel(**inputs) -> np.ndarray:
    full, _ = run(inputs, trace=False)
    return full


# revision 18
# speedup vs baseline: 1.3005x; 1.1490x over previous
"""EMA dechunker kernel for Trainium2 (Bass/Tile), 8-core data-parallel.

Problem: for each batch row
  smoothed[j] = m[j] ? clip(p[j])*emb[j] + (1-clip(p[j]))*smoothed[j-1]
                     : smoothed[j-1]
  frames[l]   = smoothed[clip(cumsum(boundary)[l]-1, 0, J-1)]

Sharding: batch dim B=16 split across 8 cores (2 rows/core). Each core:
  1. coeffs: c = clip(conf)*mask, a = 1-c  (tiny row ops)
  2. EMA: PE-transpose emb blocks (scaled by diag(c)) into (D-part, J-free)
     layout, then one tensor_tensor_scan per (row, D-block) runs the whole
     first-order recurrence along the free dim. Transpose back, store
     smoothed to DRAM.
  3. idx: two-level cumsum of boundary mask (PE tri-matmul over 16
     partitions + free-dim scan of column sums), -1, clip, cast int16.
  4. gather: dma_gather pulls frames' source rows from DRAM smoothed,
     direct DMA writes the output.
"""

from contextlib import ExitStack

import numpy as np

import concourse.bass as bass
import concourse.tile as tile
from concourse import bacc, mybir
from concourse.bass_utils import run_bass_kernel_spmd
from concourse.masks import make_identity

F32 = mybir.dt.float32
I16 = mybir.dt.int16
U8 = mybir.dt.uint8
OP = mybir.AluOpType

B, J, L, D = 16, 1024, 4096, 512
N_CORES = 8
BL = B // N_CORES          # 2 batch rows per core
T = 128                    # j-chunk (partition) size
NCH = J // T               # 8 chunks per row
NDB = D // 128             # 4 D-blocks of 128 partitions
NSUB = 8                   # sub-gathers per row
SUBL = L // NSUB           # 1024 frames per sub-gather
EPS = 1e-4


def _body(tc, ctx):
    nc = tc.nc
    emb = nc.dram_tensor("unit_embeddings", [BL, J, D], F32, kind="ExternalInput").ap()
    conf = nc.dram_tensor("unit_confidence", [BL, J], F32, kind="ExternalInput").ap()
    mask = nc.dram_tensor("unit_mask", [BL, J], U8, kind="ExternalInput").ap()
    bdry = nc.dram_tensor("boundary_mask", [BL, L], U8, kind="ExternalInput").ap()
    out = nc.dram_tensor("frames", [BL, L, D], F32, kind="ExternalOutput").ap()

    const_p = ctx.enter_context(tc.tile_pool(name="const", bufs=1))
    coef_p = ctx.enter_context(tc.tile_pool(name="coef", bufs=1))
    et_p = ctx.enter_context(tc.tile_pool(name="et", bufs=3))
    etT_p = ctx.enter_context(tc.tile_pool(name="etT", bufs=BL))
    smT_p = ctx.enter_context(tc.tile_pool(name="smT", bufs=2 * NDB))
    smn_p = ctx.enter_context(tc.tile_pool(name="smn", bufs=3))
    diag_p = ctx.enter_context(tc.tile_pool(name="diag", bufs=4))
    idx_p = ctx.enter_context(tc.tile_pool(name="idx", bufs=1))
    gout_p = ctx.enter_context(tc.tile_pool(name="gout", bufs=4))
    dram_p = ctx.enter_context(tc.tile_pool(name="dram", bufs=1, space="DRAM"))
    psum_p = ctx.enter_context(tc.tile_pool(name="psum", bufs=7, space="PSUM"))

    ps_ctr = [0]

    def ps_tile(shape):
        ps_ctr[0] += 1
        return psum_p.tile(shape, F32, tag="ps", name=f"ps{ps_ctr[0]}")

    # --- constants ---
    ident = const_p.tile([128, 128], F32)
    make_identity(nc, ident[:])
    ones_row = const_p.tile([1, 128], F32)
    nc.gpsimd.memset(ones_row[:], 1.0)
    ones_col16 = const_p.tile([16, 1], F32)
    nc.gpsimd.memset(ones_col16[:], 1.0)
    zeros_row = const_p.tile([1, 256], F32)
    nc.gpsimd.memset(zeros_row[:], 0.0)
    # tri16[k, p] = 1 iff k <= p  (lhsT for partition-dim inclusive cumsum):
    # running-sum of the identity along the free dim.
    zeros16 = const_p.tile([16, 16], F32)
    nc.gpsimd.memset(zeros16[:], 0.0)
    tri16 = const_p.tile([16, 16], F32)
    nc.vector.tensor_tensor_scan(
        out=tri16[:], data0=zeros16[:], data1=ident[:16, :16],
        initial=0.0, op0=OP.add, op1=OP.add,
    )

    smoothed = [dram_p.tile([J, D], F32, name=f"smoothed{r}") for r in range(BL)]

    # --- phase 1: coefficients ---
    c_rows = []
    a_bc = []
    for r in range(BL):
        cf = coef_p.tile([1, J], F32, tag=f"cf{r}")
        nc.sync.dma_start(cf[:], conf[r : r + 1, :])
        mk = coef_p.tile([1, J], F32, tag=f"mk{r}")
        nc.gpsimd.dma_start(mk[:], mask[r : r + 1, :])  # u8 -> f32 cast in DMA
        c_r = coef_p.tile([1, J], F32, tag=f"c{r}")
        nc.vector.tensor_scalar(
            out=c_r[:], in0=cf[:], scalar1=EPS, scalar2=1.0 - EPS,
            op0=OP.max, op1=OP.min,
        )
        nc.vector.tensor_tensor(out=c_r[:], in0=c_r[:], in1=mk[:], op=OP.mult)
        a_r = coef_p.tile([1, J], F32, tag=f"a{r}")
        nc.vector.tensor_scalar(
            out=a_r[:], in0=c_r[:], scalar1=-1.0, scalar2=1.0,
            op0=OP.mult, op1=OP.add,
        )
        c_rows.append(c_r)
        # broadcast a to 128 partitions via K=1 matmul
        abc = coef_p.tile([128, J], F32, tag=f"abc{r}")
        for h in range(J // 512):
            pb = ps_tile([128, 512])
            nc.tensor.matmul(
                out=pb[:], lhsT=ones_row[:], rhs=a_r[:, h * 512 : (h + 1) * 512],
                start=True, stop=True,
            )
            nc.scalar.copy(abc[:, h * 512 : (h + 1) * 512], pb[:])
        a_bc.append(abc)

    # c columns: cstage[(r*8+g), :] = c_r[g*128:(g+1)*128] ; transpose -> (128, 16)
    cstage = coef_p.tile([2 * NCH, T], F32)
    for r in range(BL):
        nc.sync.dma_start(cstage[r * NCH : (r + 1) * NCH, :], c_rows[r][:])
    pc = ps_tile([128, 2 * NCH])
    nc.tensor.matmul(
        out=pc[:], lhsT=cstage[:], rhs=ident[: 2 * NCH, : 2 * NCH],
        start=True, stop=True,
    )
    c_cols = coef_p.tile([128, 2 * NCH], F32)
    nc.vector.tensor_copy(c_cols[:], pc[:])



    # --- phase 2: indices ---
    idx_rep = []
    for r in range(BL):
        bd_f = idx_p.tile([1, L], F32, tag=f"bdf{r}")
        nc.gpsimd.dma_start(bd_f[:], bdry[r : r + 1, :])  # u8 -> f32
        # W[p, q] = bd[q*16 + p] for p in [0,16), q in [0,256)
        w_sb = idx_p.tile([16, 256], F32, tag=f"w{r}")
        for h in range(2):
            vh = idx_p.tile([128, 16], F32, tag=f"vh{r}")
            nc.sync.dma_start(vh[:], bd_f[:, h * 2048 : (h + 1) * 2048])
            pw = ps_tile([16, 128])
            nc.tensor.matmul(out=pw[:], lhsT=vh[:], rhs=ident[:], start=True, stop=True)
            nc.vector.tensor_copy(w_sb[:, h * 128 : (h + 1) * 128], pw[:])
        # column sums -> exclusive prefix along q
        pcs = ps_tile([1, 256])
        nc.tensor.matmul(out=pcs[:], lhsT=ones_col16[:], rhs=w_sb[:], start=True, stop=True)
        cs_sb = idx_p.tile([1, 256], F32, tag=f"cs{r}")
        nc.vector.tensor_copy(cs_sb[:], pcs[:])
        incl = idx_p.tile([1, 256], F32, tag=f"incl{r}")
        nc.vector.tensor_tensor_scan(
            out=incl[:], data0=cs_sb[:], data1=zeros_row[:],
            initial=0.0, op0=OP.add, op1=OP.add,
        )
        excl = idx_p.tile([1, 256], F32, tag=f"excl{r}")
        nc.vector.tensor_tensor(out=excl[:], in0=incl[:], in1=cs_sb[:], op=OP.subtract)
        # full cumsum = tri16 @ W + broadcast(excl)
        pidx = ps_tile([16, 256])
        nc.tensor.matmul(out=pidx[:], lhsT=tri16[:], rhs=w_sb[:], start=True, stop=False)
        nc.tensor.matmul(
            out=pidx[:], lhsT=ones_row[:, :16], rhs=excl[:], start=False, stop=True
        )
        idxf = idx_p.tile([16, 256], F32, tag=f"idxf{r}")
        nc.vector.tensor_scalar(
            out=idxf[:], in0=pidx[:], scalar1=-1.0, scalar2=0.0, op0=OP.add, op1=OP.max
        )
        nc.vector.tensor_scalar_min(idxf[:], idxf[:], float(J - 1))
        idx16 = idx_p.tile([16, 256], I16, tag=f"idx16{r}")
        nc.vector.tensor_copy(idx16[:], idxf[:])
        rep = idx_p.tile([128, 256], I16, tag=f"rep{r}")
        for k in range(8):
            nc.sync.dma_start(rep[k * 16 : (k + 1) * 16, :], idx16[:])
        idx_rep.append(rep)

    # --- phases 3+4, pipelined per batch row ---
    # eTall[r] column layout: [d*J + j] — D-block-major, j within block.
    etT = {}
    for r in range(BL):
        etT[r] = etT_p.tile([128, NDB * J], F32, tag="etT", name=f"etT{r}")

    def ema_row(r):
        for g in range(NCH):
            e_t = et_p.tile([T, D], F32, tag="et", name=f"et{r}_{g}")
            nc.sync.dma_start(e_t[:], emb[r, g * T : (g + 1) * T, :])
            # scale rows by c (per-partition scalar broadcast along D)
            col = r * NCH + g
            nc.vector.tensor_tensor(
                out=e_t[:], in0=e_t[:],
                in1=c_cols[:, col : col + 1].to_broadcast([T, D]), op=OP.mult,
            )
            pt = ps_tile([128, D])
            for d in range(NDB):
                nc.tensor.matmul(
                    out=pt[:, d * 128 : (d + 1) * 128],
                    lhsT=e_t[:, d * 128 : (d + 1) * 128],
                    rhs=ident[:], start=True, stop=True,
                    is_transpose=True,
                )
            dst = etT[r][:].rearrange("p (d j) -> p d j", d=NDB)[
                :, :, g * T : (g + 1) * T
            ]
            src = pt[:].rearrange("p (d j) -> p d j", d=NDB)
            if g % 2 == 0:
                nc.vector.tensor_copy(dst, src)
            else:
                nc.scalar.copy(dst, src)

        smT = {}
        for d in range(NDB):
            st = smT_p.tile([128, J], F32, tag="smT", name=f"smT{r}_{d}")
            nc.vector.tensor_tensor_scan(
                out=st[:], data0=a_bc[r][:],
                data1=etT[r][:, d * J : (d + 1) * J],
                initial=0.0, op0=OP.mult, op1=OP.add,
            )
            smT[d] = st

        for g in range(NCH):
            smn = smn_p.tile([T, D], F32, tag="smn", name=f"smn{r}_{g}")
            pt2 = ps_tile([128, D])
            for d in range(NDB):
                nc.tensor.matmul(
                    out=pt2[:, d * 128 : (d + 1) * 128],
                    lhsT=smT[d][:, g * T : (g + 1) * T],
                    rhs=ident[:], start=True, stop=True, is_transpose=True,
                )
            if g % 2 == 0:
                nc.vector.tensor_copy(smn[:], pt2[:])
            else:
                nc.scalar.copy(smn[:], pt2[:])
            nc.scalar.dma_start(smoothed[r][g * T : (g + 1) * T, :], smn[:])

    def gather_row(r):
        for s in range(NSUB):
            gt = gout_p.tile([128, SUBL // 128, D], F32, tag="gout", name=f"gout{r}_{s}")
            nc.gpsimd.dma_gather(
                out_ap=gt[:],
                in_ap=smoothed[r][:],
                idxs_ap=idx_rep[r][:, s * (SUBL // 16) : (s + 1) * (SUBL // 16)],
                num_idxs=SUBL,
                num_idxs_reg=SUBL,
                elem_size=D,
                queue_num=s % 2,
            )
            dst = out[r, s * SUBL : (s + 1) * SUBL, :].rearrange(
                "(g p) d -> p g d", p=128
            )
            if s % 2 == 0:
                nc.sync.dma_start(dst, gt[:])
            else:
                nc.scalar.dma_start(dst, gt[:])

    ema_row(0)
    gather_row(0)
    ema_row(1)
    gather_row(1)


def _patch_swdge_lane_by_queue():
    """Tile assigns DMASW completion-sem lanes round-robin, queue-blind; the
    HW/sim lock each lane's sem to one SWDGE queue. Pin lane = queue_num so
    multi-queue gathers get consistent lanes."""
    from concourse import bass_isa
    from concourse import tile_sem_assignment as tsa

    if getattr(tsa.TileClockTick, "_ema_queue_patch", False):
        return
    orig = tsa.TileClockTick._assign_tick

    def patched(self, inst):
        if (
            isinstance(inst, bass_isa.AnyDMAInstruction)
            and inst.engine == mybir.EngineType.Pool
            and not isinstance(inst, bass_isa.UserSyncedRemoteDMADescs)
        ):
            self.next_sw_dma_idx = getattr(inst, "queue_num", 0) or 0
        return orig(self, inst)

    tsa.TileClockTick._assign_tick = patched
    tsa.TileClockTick._ema_queue_patch = True


def build():
    _patch_swdge_lane_by_queue()
    nc = bacc.Bacc(
        "TRN2",
        target_bir_lowering=False,
        debug=False,
        enable_asserts=False,
        num_devices=N_CORES,
        num_swdge_queues=2,
        dynamic_dma_scratch_size=28672,
    )
    with tile.TileContext(nc) as tc, ExitStack() as ctx:
        _body(tc, ctx)
    nc.compile()
    return nc


def make_in_maps(inputs):
    emb = np.asarray(inputs["unit_embeddings"], dtype=np.float32)
    conf = np.asarray(inputs["unit_confidence"], dtype=np.float32)
    msk = np.asarray(inputs["unit_mask"]).astype(np.uint8)
    bd = np.asarray(inputs["boundary_mask"]).astype(np.uint8)
    in_maps = []
    for c in range(N_CORES):
        sl = slice(c * BL, (c + 1) * BL)
        in_maps.append(
            {
                "unit_embeddings": np.ascontiguousarray(emb[sl]),
                "unit_confidence": np.ascontiguousarray(conf[sl]),
                "unit_mask": np.ascontiguousarray(msk[sl]),
                "boundary_mask": np.ascontiguousarray(bd[sl]),
            }
        )
    return in_maps


_cached_nc = None


def run(inputs, trace=False):
    global _cached_nc
    if _cached_nc is None:
        _cached_nc = build()
    res = run_bass_kernel_spmd(
        _cached_nc, make_in_maps(inputs), core_ids=list(range(N_CORES)), trace=trace
    )
    full = np.concatenate(
        [res.results[c]["frames"] for c in range(N_CORES)], axis=0
    )
    return full, res


def kernel(**inputs) -> np.ndarray:
    full, _ = run(inputs, trace=False)
    return full
